# revision 21
# baseline (speedup 1.0000x reference)
"""Multi-head attention (B=4, T=2048, H=1024, nh=16) on 8 Trainium2 cores.

Sharding: core = (batch b, head-group g); 4 batches x 2 groups of 8 heads.
Each core computes Q^T/K^T projections for its 512 head-dims, the V
projection (shipped to HBM), and per head the softmax-weighted column
means cbar[s] = sum_t exp(scores[t,s])/denom[t].  Because the reference
takes mean over T before the output projection, the full [T,T]x[T,dh]
context matmul collapses: ctx_mean[d] = (1/T) sum_s cbar[s] V[s,d],
which the host finishes along with the (tiny) Wo projection.

cbar is computed with w-stationary matmuls: per 128-row tile of the exp
matrix w, 16 matmuls with lhsT = w[:, sb*128:(sb+1)*128] (the stationary
weight) and rhs = r = 1/denom [128, 1] accumulate cb[sp, sb] over the 16
t-tiles of the head.  This streams only 1 column per matmul (vs 2048 for
the r-stationary form), trading PE stream time for weight loads.
"""

import numpy as np

B, T, C = 4, 2048, 1024
NH, DH = 16, 64
HLOC = 8          # heads per core
D = HLOC * DH     # 512 projection dims per core
N_CORES = 8

C_TILES = C // 128    # 8
T_TILES = T // 128    # 16
S_BLOCKS = T // 128   # 16 cbar column blocks

_CACHE = {}
TRACE = False
TRACE_KWARGS = {}


def _build():
    import concourse.mybir as mybir
    import concourse.tile as tile
    from concourse import bacc

    f32 = mybir.dt.float32
    f32r = mybir.dt.float32r
    bf16 = mybir.dt.bfloat16
    Exp = mybir.ActivationFunctionType.Exp

    nc = bacc.Bacc("TRN2", target_bir_lowering=False, debug=False,
                   num_devices=N_CORES)

    XT = nc.dram_tensor("xT", [C, T], f32, kind="ExternalInput").ap()
    WQT = nc.dram_tensor("wqT", [C, D], f32, kind="ExternalInput").ap()
    WKT = nc.dram_tensor("wkT", [C, D], f32, kind="ExternalInput").ap()
    WVT = nc.dram_tensor("wvT", [C, D], f32, kind="ExternalInput").ap()
    # cbar, transposed: [head, s_part(128), s_block(16)], s = sb*128 + sp
    CBAR = nc.dram_tensor("cbar", [HLOC, 128, S_BLOCKS], f32,
                          kind="ExternalOutput").ap()
    VOUT = nc.dram_tensor("vout", [T, D], f32, kind="ExternalOutput").ap()

    with tile.TileContext(nc) as tc, \
         nc.allow_low_precision("float32r tags carry full fp32 bits"):
        with tc.tile_pool(name="load", bufs=1) as load, \
             tc.tile_pool(name="qtkt", bufs=2) as qtkt, \
             tc.tile_pool(name="wpool", bufs=3) as wpool, \
             tc.tile_pool(name="small", bufs=4) as small, \
             tc.tile_pool(name="stage", bufs=2) as stage_pool, \
             tc.tile_pool(name="vstage", bufs=2) as vstage:

            xt_all = load.tile([128, C_TILES * T], f32r)
            wq_all = load.tile([128, C_TILES * D], f32r)
            wk_all = load.tile([128, C_TILES * D], f32r)
            wv_all = load.tile([128, C_TILES * D], f32r)
            # DMA order by first-use time: the pair-0 projections need only
            # the dt=0 column slice (128 of 512 cols) of wq/wk plus all of
            # xt, so those go first; the remaining wq/wk cols are consumed
            # by the boundary-window projections (heads 0+), wv last (V
            # projection runs inside heads 2+).
            # all 8 dt0 column slices of wk/wq in ONE strided DMA each (the
            # prefix K and Q-tb0 chains need only these); dst free dims
            # [c-block, col], src blocks [c][row][col]
            nc.scalar.dma_start(
                wk_all.rearrange("p (c d) -> p c d", c=C_TILES)[:, :, 0:128],
                WKT.rearrange("(c p) d -> p c d", c=C_TILES)[:, :, 0:128]
                .bitcast(f32r))
            nc.sync.dma_start(
                wq_all.rearrange("p (c d) -> p c d", c=C_TILES)[:, :, 0:128],
                WQT.rearrange("(c p) d -> p c d", c=C_TILES)[:, :, 0:128]
                .bitcast(f32r))
            for c in range(C_TILES):
                eng_w = nc.scalar if c % 2 == 0 else nc.sync
                eng_x = nc.sync if c % 2 == 0 else nc.scalar
                # xt c-tile split across both queues to halve its landing time
                eng_x.dma_start(xt_all[:, c * T: c * T + 1024],
                               XT[c * 128:(c + 1) * 128, 0:1024].bitcast(f32r))
                eng_w.dma_start(xt_all[:, c * T + 1024: (c + 1) * T],
                               XT[c * 128:(c + 1) * 128, 1024:2048].bitcast(f32r))
            for c in range(C_TILES):
                eng_w = nc.scalar if c % 2 == 0 else nc.sync
                eng_x = nc.sync if c % 2 == 0 else nc.scalar
                eng_w.dma_start(wq_all[:, c * D + 128:(c + 1) * D],
                                WQT[c * 128:(c + 1) * 128, 128:D].bitcast(f32r))
                eng_x.dma_start(wk_all[:, c * D + 128:(c + 1) * D],
                                WKT[c * 128:(c + 1) * 128, 128:D].bitcast(f32r))
            for c in range(C_TILES):
                nc.sync.dma_start(wv_all[:, c * D:(c + 1) * D],
                                  WVT[c * 128:(c + 1) * 128, :].bitcast(f32r))

            # per-pair Q^T/K^T tiles [128, T]; rows = 2 heads x 64 dims
            qt, kt = {}, {}

            def alloc_pair(p):
                # bf16: scores-matmul weight reloads qualify for FWL (4x
                # faster load; disabled for fp32/f32r), ~0.5% scores error
                qt[p] = qtkt.tile([128, T], bf16, tag="qt", name=f"qt{p}")
                kt[p] = qtkt.tile([128, T], bf16, tag="kt", name=f"kt{p}")

            def proj_mm(psum_ap, w_all, dt_, tb, c):
                nc.tensor.matmul(
                    psum_ap,
                    w_all[:, c * D + dt_ * 128: c * D + (dt_ + 1) * 128],
                    xt_all[:, c * T + tb * 512: c * T + tb * 512 + 512],
                    start=(c == 0), stop=(c == C_TILES - 1))

            # ---- prefix: K^T (all) + Q^T (first 512 t-cols) for pair 0 ----
            # Head-0 scores need ALL of kt[0] but only qt[0] cols covering
            # the current t-tile, so the prefix computes just Q tb=0; the Q
            # tb=1..3 chains run in head-0 boundary windows (each ready
            # well before its t-tiles come up).  This minimizes the time to
            # the first exp, which gates the whole ACT stream.
            alloc_pair(0)
            # touch the exp table set early so its ~2.7us load overlaps DMA
            preheat = small.tile([128, 1], f32, tag="preheat")
            nc.gpsimd.memset(preheat[:], 0.0)
            nc.scalar.activation(preheat[:], preheat[:], Exp)
            with tc.tile_pool(name="proj_ps", bufs=1, space="PSUM") as proj_ps:
                pk = [proj_ps.tile([128, 512], f32, tag=f"ppk{i}", bufs=1,
                                   name=f"ppk{i}") for i in range(4)]
                pq0 = proj_ps.tile([128, 512], f32, tag="ppq0", bufs=1,
                                   name="ppq0")
                # PE warmup: dummy matmuls on memset data keep the PE busy
                # from t~0 so the HAM clock-gate ramps to full speed before
                # the DMA-gated projection chains start (PE would otherwise
                # run the whole prefix at the cold 1.2 GHz rate).
                warm_w = small.tile([128, 128], bf16, tag="warm_w")
                warm_x = small.tile([128, 512], bf16, tag="warm_x")
                nc.vector.memset(warm_w[:], 0.0)
                nc.vector.memset(warm_x[:], 0.0)
                pwarm = proj_ps.tile([128, 512], f32, tag="pwarm", bufs=1,
                                     name="pwarm")
                for i in range(18):
                    nc.tensor.matmul(pwarm[:], warm_w[:], warm_x[:],
                                     start=True, stop=True)
                for c in range(C_TILES):
                    for tb in range(4):
                        proj_mm(pk[tb][:], wk_all, 0, tb, c)
                for c in range(C_TILES):
                    proj_mm(pq0[:], wq_all, 0, 0, c)
                # copy order = first-use order of head-0's first exp: the
                # first activation reads kt tb0/tb1 + qt tb0 only
                nc.vector.tensor_copy(kt[0][:, 0:512], pk[0][:])
                nc.vector.tensor_copy(kt[0][:, 512:1024], pk[1][:])
                nc.vector.tensor_copy(qt[0][:, 0:512], pq0[:])
                nc.vector.tensor_copy(kt[0][:, 1024:1536], pk[2][:])
                nc.vector.tensor_copy(kt[0][:, 1536:2048], pk[3][:])
                alloc_pair(1)

            # ---- attention ----
            with tc.tile_pool(name="score_ps", bufs=2, space="PSUM") as score_ps, \
                 tc.tile_pool(name="cb_ps", bufs=2, space="PSUM") as cb_ps, \
                 tc.tile_pool(name="aux_ps", bufs=2, space="PSUM") as aux_ps:

                def qtkt_group(w_all, dst, dt_, tb):
                    pj = aux_ps.tile([128, 512], f32, tag="aux", name="pj")
                    for c in range(C_TILES):
                        proj_mm(pj[:], w_all, dt_, tb, c)
                    nc.vector.tensor_copy(dst[:, tb * 512:(tb + 1) * 512],
                                          pj[:])

                def v_group(tt_v):
                    pj = aux_ps.tile([128, 512], f32, tag="aux", name="pv2")
                    for c in range(C_TILES):
                        nc.tensor.matmul(
                            pj[:],
                            xt_all[:, c * T + tt_v * 128: c * T + (tt_v + 1) * 128],
                            wv_all[:, c * D:(c + 1) * D],
                            start=(c == 0), stop=(c == C_TILES - 1))
                    vs = vstage.tile([128, D], f32)
                    nc.vector.tensor_copy(vs[:], pj[:])
                    nc.sync.dma_start(VOUT[tt_v * 128:(tt_v + 1) * 128, :], vs[:])

                def mk_qtkt(w_all_n, p_n, tb):
                    return lambda _w=w_all_n, _p=p_n, _tb=tb: qtkt_group(
                        _w, (qt if _w is wq_all else kt)[_p], _p, _tb)

                def mk_v(tt_v):
                    return lambda _t=tt_v: v_group(_t)

                # per-head window work (projections for later pairs + the V
                # projection), one group per t-tile slot; None = empty slot.
                # Head 0 finishes its own Q tb=1..3 (deferred from the
                # prefix; tb ready by slot tb+1 < its first use at tt=4*tb).
                boundary_sched = {
                    0: [None] +
                       [mk_qtkt(wq_all, 0, tb) for tb in (1, 2, 3)] +
                       [mk_qtkt(wq_all, 1, tb) for tb in range(4)],
                    1: [mk_qtkt(wk_all, 1, tb) for tb in range(4)] +
                       [mk_v(0), mk_v(1)],
                    2: [mk_qtkt(wq_all, 2, tb) for tb in range(4)] +
                       [mk_v(2), mk_v(3)],
                    3: [mk_qtkt(wk_all, 2, tb) for tb in range(4)] +
                       [mk_v(4), mk_v(5)],
                    4: [mk_qtkt(wq_all, 3, tb) for tb in range(4)] +
                       [mk_v(6), mk_v(7)],
                    5: [mk_qtkt(wk_all, 3, tb) for tb in range(4)] +
                       [mk_v(8), mk_v(9)],
                    6: [mk_v(10 + i) for i in range(3)],
                    7: [mk_v(13 + i) for i in range(3)],
                }

                for H in range(HLOC):
                    pair = H // 2
                    odd = H % 2
                    row0 = 64 * odd
                    if pair in (1, 2) and odd == 0:
                        alloc_pair(pair + 1)
                    cb = cb_ps.tile([128, S_BLOCKS], f32, tag="cb", name="cb")
                    for tt in range(T_TILES):
                        qs = qt[pair][row0:row0 + 64, tt * 128:(tt + 1) * 128]
                        sc = [score_ps.tile([128, 1024], f32, tag="sc",
                                            name=f"sc{i}") for i in range(2)]
                        for i in range(2):
                            for j in range(2):
                                s_blk = i * 2 + j
                                nc.tensor.matmul(
                                    sc[i][:, j * 512:(j + 1) * 512],
                                    qs,
                                    kt[pair][row0:row0 + 64,
                                             s_blk * 512: s_blk * 512 + 512],
                                    start=True, stop=True)
                        wk_sched = boundary_sched[H]
                        if tt < len(wk_sched) and wk_sched[tt] is not None:
                            wk_sched[tt]()

                        w = wpool.tile([128, T], bf16)
                        accs = small.tile([128, 2], f32, tag="accs")
                        for i in range(2):
                            nc.scalar.activation(
                                w[:, i * 1024:(i + 1) * 1024], sc[i][:], Exp,
                                scale=0.125, accum_out=accs[:, i:i + 1])
                        denom = small.tile([128, 1], f32, tag="denom")
                        nc.vector.tensor_add(denom[:], accs[:, 0:1],
                                             accs[:, 1:2])
                        r32 = small.tile([128, 1], f32, tag="r32")
                        nc.vector.reciprocal(r32[:], denom[:])
                        r = small.tile([128, 1], bf16, tag="r")
                        nc.vector.tensor_copy(r[:], r32[:])
                        # all 256 MMs of the head form ONE psum accumulation
                        # group (zero regions are bank-granular): start clears
                        # the bank's has_written bits, so each column's first
                        # write lands in overwrite mode, later ones accumulate
                        for sb in range(S_BLOCKS):
                            nc.tensor.matmul(
                                cb[:, sb:sb + 1],
                                w[:, sb * 128:(sb + 1) * 128],
                                r[:],
                                start=(tt == 0 and sb == 0),
                                stop=(tt == T_TILES - 1 and sb == S_BLOCKS - 1))
                    stg = stage_pool.tile([128, S_BLOCKS], f32)
                    nc.vector.tensor_copy(stg[:], cb[:])
                    nc.sync.dma_start(CBAR[H, :, :], stg[:])

    nc.compile()
    return nc


def _setup_exec(cache=None, **build_kwargs):
    """Build the Bass module and a cached jitted SPMD executor
    (mirrors concourse.bass2jax.run_bass_via_pjrt's multi-core path)."""
    import jax
    import concourse.mybir as mybir
    from concourse import bass2jax
    from jax.experimental.shard_map import shard_map
    from jax.sharding import Mesh, PartitionSpec

    if cache is None:
        cache = _CACHE
    nc = _build(**build_kwargs)
    bass2jax.install_neuronx_cc_hook()

    partition_name = (nc.partition_id_tensor.name
                      if nc.partition_id_tensor else None)
    in_names, out_names, out_avals, zero_shapes = [], [], [], []
    for alloc in nc.m.functions[0].allocations:
        if not isinstance(alloc, mybir.MemoryLocationSet):
            continue
        name = alloc.memorylocations[0].name
        if alloc.kind == "ExternalInput":
            if name != partition_name:
                in_names.append(name)
        elif alloc.kind == "ExternalOutput":
            shape = tuple(alloc.tensor_shape)
            dtype = mybir.dt.np(alloc.dtype)
            out_names.append(name)
            out_avals.append(jax.core.ShapedArray(shape, dtype))
            zero_shapes.append((shape, dtype))
    n_params = len(in_names)
    all_in_names = in_names + out_names
    if partition_name is not None:
        all_in_names = all_in_names + [partition_name]

    def _body(*args):
        operands = list(args)
        if partition_name is not None:
            operands.append(bass2jax.partition_id_tensor())
        outs = bass2jax._bass_exec_p.bind(
            *operands,
            out_avals=tuple(out_avals),
            in_names=tuple(all_in_names),
            out_names=tuple(out_names),
            lowering_input_output_aliases=(),
            sim_require_finite=True,
            sim_require_nnan=True,
            nc=nc,
        )
        return tuple(outs)

    devices = jax.devices()[:N_CORES]
    mesh = Mesh(np.asarray(devices), ("core",))
    n_outs = len(out_names)
    sharded = jax.jit(
        shard_map(_body, mesh=mesh,
                  in_specs=(PartitionSpec("core"),) * (n_params + n_outs),
                  out_specs=(PartitionSpec("core"),) * n_outs,
                  check_rep=False),
        donate_argnums=tuple(range(n_params, n_params + n_outs)),
        keep_unused=True,
    )

    from jax.sharding import NamedSharding
    shardings = NamedSharding(mesh, PartitionSpec("core"))

    def make_zeros():
        import jax.numpy as jnp
        return [
            jax.device_put(
                jnp.zeros((N_CORES * s[0], *s[1:]), d), shardings)
            for s, d in zero_shapes
        ]

    cache.update(nc=nc, sharded=sharded, in_names=in_names,
                 out_names=out_names, out_avals=out_avals,
                 make_zeros=make_zeros, shardings=shardings)
    return cache


def kernel(x, Wq, Wk, Wv, Wo, bo):
    import jax

    x = np.asarray(x, dtype=np.float32)
    Wq = np.asarray(Wq, dtype=np.float32)
    Wk = np.asarray(Wk, dtype=np.float32)
    Wv = np.asarray(Wv, dtype=np.float32)
    Wo = np.asarray(Wo, dtype=np.float32)
    bo = np.asarray(bo, dtype=np.float32)

    if "sharded" not in _CACHE:
        _setup_exec()

    in_maps = []
    for b in range(B):
        xtb = np.ascontiguousarray(x[b].T)            # [C, T]
        for g in range(2):
            rows = slice(g * D, (g + 1) * D)
            in_maps.append({
                "xT": xtb,
                "wqT": np.ascontiguousarray(Wq[rows, :].T),
                "wkT": np.ascontiguousarray(Wk[rows, :].T),
                "wvT": np.ascontiguousarray(Wv[rows, :].T),
            })

    concat_in = [
        np.concatenate([in_maps[c][name] for c in range(N_CORES)], axis=0)
        for name in _CACHE["in_names"]
    ]
    device_inputs = [jax.device_put(a, _CACHE["shardings"]) for a in concat_in]
    _CACHE["device_inputs"] = device_inputs

    out_arrs = _CACHE["sharded"](*device_inputs, *_CACHE["make_zeros"]())
    results = [
        {name: np.asarray(out_arrs[i]).reshape(N_CORES, *_CACHE["out_avals"][i].shape)[c]
         for i, name in enumerate(_CACHE["out_names"])}
        for c in range(N_CORES)
    ]

    ctx_mean = np.empty((B, C), dtype=np.float32)
    for core in range(N_CORES):
        b, g = divmod(core, 2)
        cbt = results[core]["cbar"]                   # [8, 128, 16]
        # s = sb*128 + sp -> cbar[h, s]
        cbar = cbt.transpose(0, 2, 1).reshape(HLOC, T)
        vout = results[core]["vout"]                  # [T, 512]
        v_r = vout.reshape(T, HLOC, DH)
        cm = np.einsum("hs,shd->hd", cbar, v_r, optimize=True) / np.float32(T)
        ctx_mean[b, g * D:(g + 1) * D] = cm.reshape(-1)

    return ctx_mean @ Wo.T + bo
